# revision 8
# baseline (speedup 1.0000x reference)
"""Causal single-head attention (S=4096, D=1024, fp32) on 8 TRN2 NeuronCores.

Sharding: q rows are sharded across the 8 cores (512 rows each); k/v inputs
and weights are replicated (each core computes the full K/V projections).

Host-side prep (part of the sharding strategy): inputs are pre-transposed
and cast to bf16 so that every on-device matmul has its contraction dim on
SBUF partitions with zero on-device transposes:

  qT[o,i]  = matmul(lhsT=WqT[d,o],  rhs=xqT[d,i])      (PSUM -> bf16 SBUF)
  kT[o,j]  = matmul(lhsT=WkT[d,o]/sqrt(D), rhs=xkT[d,j])
  V[j,o]   = matmul(lhsT=xvT[d,j],  rhs=WvT[d,o])
  ST[j,i]  = matmul(lhsT=kT[o,j],   rhs=qT[o,i])       (scores, transposed)
  PT[j,i]  = exp(ST) * causal01[j,i]                   (ACT exp + DVE mask)
  O[i,o]   = sum_j matmul(lhsT=PT[j,i], rhs=V[j,o])    (PSUM accum over j)
  sums[i]  = sum_j matmul(lhsT=PT[j,i], rhs=ones[j,1])
  out[i,o] = O[i,o] / sums[i]

Softmax skips max-subtraction: scores are q.k/sqrt(D) with |s| <~ 4 for this
problem family, far from fp32 exp overflow, and the reference's max-shift
cancels exactly in exact arithmetic.
"""

import numpy as np
import ml_dtypes

import concourse.bacc as bacc
import concourse.tile as tile
from concourse import mybir
from concourse.bass_utils import run_bass_kernel_spmd

S = 4096
D = 1024
NCORES = 8
ROWS = S // NCORES  # 512 q rows per core
P = 128
DC = D // P   # 8 contraction chunks
OT = D // P   # 8 output-dim tiles
NJB = 8       # 512-wide j blocks for projections
NJT = S // P  # 32 j tiles of 128
NIB = 2       # i blocks per core
IB = ROWS // NIB  # 256
BF = mybir.dt.bfloat16
F32 = mybir.dt.float32
EXP = mybir.ActivationFunctionType.Exp

bf16 = ml_dtypes.bfloat16

# debugging knob: "full" | "proj" (stop after projections) | "scores"
# (projections + S^T/exp/mask) | "nosums" (full minus the N=1 sums matmuls)
VARIANT = "full"


def build_nc():
    variant = VARIANT
    nc = bacc.Bacc(None, target_bir_lowering=False, debug=False)

    xq = nc.declare_dram_parameter("xqt", [D, ROWS], BF, isOutput=False)
    xk = nc.declare_dram_parameter("xkt", [D, S], BF, isOutput=False)
    xv = nc.declare_dram_parameter("xvt", [D, S], BF, isOutput=False)
    wq = nc.declare_dram_parameter("wqt", [D, D], BF, isOutput=False)
    wk = nc.declare_dram_parameter("wkt", [D, D], BF, isOutput=False)
    wv = nc.declare_dram_parameter("wvt", [D, D], BF, isOutput=False)
    msk = nc.declare_dram_parameter("mask01", [NIB * NJT, P, IB], BF, isOutput=False)
    out = nc.declare_dram_parameter("out", [ROWS, D], F32, isOutput=True)

    with tile.TileContext(nc) as tc:
        with tc.tile_pool(name="persist", bufs=1) as persist:
            ones = persist.tile([P, 16], BF, tag="ones", name="ones")
            nc.vector.memset(ones[:], 1.0)
            zbias = persist.tile([P, 1], F32, tag="zbias", name="zbias")
            nc.vector.memset(zbias[:], 0.0)
            qT = [persist.tile([P, ROWS], BF, tag=f"qT{t}", name=f"qT{t}") for t in range(OT)]
            kT = [persist.tile([P, S], BF, tag=f"kT{t}", name=f"kT{t}") for t in range(OT)]
            Vt = [persist.tile([P, D], BF, tag=f"V{j}", name=f"V{j}") for j in range(NJT)]

            # ---- Q projection ----
            with (
                tc.tile_pool(name="qph", bufs=1) as qp,
                tc.tile_pool(name="pps_q", bufs=3, space="PSUM") as pps,
            ):
                xq_t = [qp.tile([P, ROWS], BF, tag=f"xq{d_}", name=f"xq{d_}") for d_ in range(DC)]
                wq_t = [qp.tile([P, D], BF, tag=f"wq{d_}", name=f"wq{d_}") for d_ in range(DC)]
                for d_ in range(DC):
                    nc.sync.dma_start(out=xq_t[d_][:], in_=xq[d_ * P:(d_ + 1) * P, :])
                    nc.sync.dma_start(out=wq_t[d_][:], in_=wq[d_ * P:(d_ + 1) * P, :])
                for t in range(OT):
                    ps = pps.tile([P, ROWS], F32, tag="pp", name="ppq")
                    for d_ in range(DC):
                        nc.tensor.matmul(
                            ps[:],
                            lhsT=wq_t[d_][:, t * P:(t + 1) * P],
                            rhs=xq_t[d_][:],
                            start=(d_ == 0),
                            stop=(d_ == DC - 1),
                        )
                    nc.vector.tensor_copy(qT[t][:], ps[:])

            # ---- K projection (kT, scale folded into wk on host) ----
            with (
                tc.tile_pool(name="kph_w", bufs=1) as wpk,
                tc.tile_pool(name="kph_x", bufs=2) as xpk,
                tc.tile_pool(name="pps_k", bufs=4, space="PSUM") as pps,
            ):
                wk_t = [wpk.tile([P, D], BF, tag=f"wk{d_}", name=f"wk{d_}") for d_ in range(DC)]
                for d_ in range(DC):
                    nc.sync.dma_start(out=wk_t[d_][:], in_=wk[d_ * P:(d_ + 1) * P, :])
                for jb in range(NJB):
                    xkb = xpk.tile([P, DC, 512], BF, tag="xkb", name="xkb")
                    for d_ in range(DC):
                        nc.sync.dma_start(
                            out=xkb[:, d_, :],
                            in_=xk[d_ * P:(d_ + 1) * P, jb * 512:(jb + 1) * 512],
                        )
                    for t in range(OT):
                        ps = pps.tile([P, 512], F32, tag="pp", name="pp")
                        for d_ in range(DC):
                            nc.tensor.matmul(
                                ps[:],
                                lhsT=wk_t[d_][:, t * P:(t + 1) * P],
                                rhs=xkb[:, d_, :],
                                start=(d_ == 0),
                                stop=(d_ == DC - 1),
                            )
                        nc.vector.tensor_copy(
                            kT[t][:, jb * 512:(jb + 1) * 512], ps[:]
                        )

            # ---- V projection (natural [j, o] layout) ----
            with (
                tc.tile_pool(name="vph_w", bufs=1) as wpv,
                tc.tile_pool(name="vph_x", bufs=2) as xpv,
                tc.tile_pool(name="pps_v", bufs=4, space="PSUM") as pps,
            ):
                wv_t = [wpv.tile([P, D], BF, tag=f"wv{d_}", name=f"wv{d_}") for d_ in range(DC)]
                for d_ in range(DC):
                    nc.sync.dma_start(out=wv_t[d_][:], in_=wv[d_ * P:(d_ + 1) * P, :])
                for jb in range(NJB):
                    xvb = xpv.tile([P, DC, 512], BF, tag="xvb", name="xvb")
                    for d_ in range(DC):
                        nc.sync.dma_start(
                            out=xvb[:, d_, :],
                            in_=xv[d_ * P:(d_ + 1) * P, jb * 512:(jb + 1) * 512],
                        )
                    for js in range(4):
                        j = 4 * jb + js
                        for ob in range(2):
                            ps = pps.tile([P, 512], F32, tag="pp", name="pp")
                            for d_ in range(DC):
                                nc.tensor.matmul(
                                    ps[:],
                                    lhsT=xvb[:, d_, js * P:(js + 1) * P],
                                    rhs=wv_t[d_][:, ob * 512:(ob + 1) * 512],
                                    start=(d_ == 0),
                                    stop=(d_ == DC - 1),
                                )
                            nc.vector.tensor_copy(
                                Vt[j][:, ob * 512:(ob + 1) * 512], ps[:]
                            )

            if variant == "proj":
                # dump a qT tile so the output is produced; skip attention
                with tc.tile_pool(name="dbg", bufs=1) as dbg:
                    for t in range(2):
                        o = dbg.tile([P, ROWS], F32, tag="dbgo", name="dbgo")
                        nc.vector.tensor_copy(o[:], qT[t][:])
                        nc.sync.dma_start(out=out[t * P:(t + 1) * P, :ROWS], in_=o[:])
                return nc

            # ---- Attention ----
            with (
                tc.tile_pool(name="att", bufs=4) as ap,
                tc.tile_pool(name="att_out", bufs=4) as op,
                tc.tile_pool(name="sps", bufs=2, space="PSUM") as spsum,
                tc.tile_pool(name="ops", bufs=1, space="PSUM") as opsum,
            ):
                for ib in range(NIB):
                    ops = {
                        (isub, ob): opsum.tile([P, 512], F32, tag=f"o{isub}{ob}", name=f"o{isub}{ob}")
                        for isub in range(2)
                        for ob in range(2)
                    }
                    sums = {
                        isub: opsum.tile([P, 16], F32, tag=f"s{isub}", name=f"s{isub}")
                        for isub in range(2)
                    }
                    for jt in range(NJT):
                        sp = spsum.tile([P, IB], F32, tag="sps", name="sps")
                        for oc in range(OT):
                            nc.tensor.matmul(
                                sp[:],
                                lhsT=kT[oc][:, jt * P:(jt + 1) * P],
                                rhs=qT[oc][:, ib * IB:(ib + 1) * IB],
                                start=(oc == 0),
                                stop=(oc == OT - 1),
                            )
                        pt = ap.tile([P, IB], BF, tag="pt", name="pt")
                        nc.scalar.activation(pt[:], sp[:], EXP, bias=zbias[:])
                        mt = ap.tile([P, IB], BF, tag="mt", name="mt")
                        nc.sync.dma_start(out=mt[:], in_=msk[ib * NJT + jt, :, :])
                        nc.vector.tensor_mul(pt[:], pt[:], mt[:])
                        if variant == "scores":
                            continue
                        for isub in range(2):
                            pslice = pt[:, isub * P:(isub + 1) * P]
                            for ob in range(2):
                                nc.tensor.matmul(
                                    ops[(isub, ob)][:],
                                    lhsT=pslice,
                                    rhs=Vt[jt][:, ob * 512:(ob + 1) * 512],
                                    start=(jt == 0),
                                    stop=(jt == NJT - 1),
                                )
                            if variant != "nosums":
                                nc.tensor.matmul(
                                    sums[isub][:],
                                    lhsT=pslice,
                                    rhs=ones[:],
                                    start=(jt == 0),
                                    stop=(jt == NJT - 1),
                                )
                    if variant == "scores":
                        continue
                    for isub in range(2):
                        rec = op.tile([P, 1], F32, tag="rec", name="rec")
                        if variant == "nosums":
                            nc.vector.memset(rec[:], 1.0)
                        else:
                            ssb = op.tile([P, 1], F32, tag="ssb", name="ssb")
                            nc.vector.tensor_copy(ssb[:], sums[isub][:, 0:1])
                            nc.vector.reciprocal(rec[:], ssb[:])
                        r0 = ib * IB + isub * P
                        for ob in range(2):
                            osb = op.tile([P, 512], F32, tag="osb", name="osb")
                            nc.vector.tensor_scalar_mul(
                                osb[:], ops[(isub, ob)][:], rec[:]
                            )
                            nc.sync.dma_start(
                                out=out[r0:r0 + P, ob * 512:(ob + 1) * 512],
                                in_=osb[:],
                            )
    return nc


_CACHE = {}


def _get_nc():
    if "nc" not in _CACHE:
        nc = build_nc()
        nc.compile()
        _CACHE["nc"] = nc
    return _CACHE["nc"]


def build_in_maps(inputs):
    x_q = np.asarray(inputs["encodings_for_q"], dtype=np.float32)
    x_k = np.asarray(inputs["encodings_for_k"], dtype=np.float32)
    x_v = np.asarray(inputs["encodings_for_v"], dtype=np.float32)
    W_q = np.asarray(inputs["W_q"], dtype=np.float32)
    W_k = np.asarray(inputs["W_k"], dtype=np.float32)
    W_v = np.asarray(inputs["W_v"], dtype=np.float32)

    xkt = np.ascontiguousarray(x_k.T).astype(bf16)
    xvt = np.ascontiguousarray(x_v.T).astype(bf16)
    wqt = np.ascontiguousarray(W_q.T).astype(bf16)
    wkt = np.ascontiguousarray(W_k.T / np.sqrt(D)).astype(bf16)
    wvt = np.ascontiguousarray(W_v.T).astype(bf16)

    causal = (np.arange(S)[:, None] <= np.arange(S)[None, :])

    in_maps = []
    for c in range(NCORES):
        rows = slice(ROWS * c, ROWS * (c + 1))
        xqt_c = np.ascontiguousarray(x_q[rows].T).astype(bf16)
        m = causal[:, rows]  # [S, ROWS]
        mask_c = np.ascontiguousarray(
            m.reshape(NJT, P, NIB, IB).transpose(2, 0, 1, 3).reshape(NIB * NJT, P, IB)
        ).astype(bf16)
        in_maps.append(
            dict(
                xqt=xqt_c, xkt=xkt, xvt=xvt,
                wqt=wqt, wkt=wkt, wvt=wvt,
                mask01=mask_c,
            )
        )
    return in_maps


def kernel(**inputs):
    nc = _get_nc()
    in_maps = build_in_maps(inputs)
    res = run_bass_kernel_spmd(nc, in_maps, list(range(NCORES)))
    outs = [np.asarray(res.results[i]["out"], dtype=np.float32) for i in range(NCORES)]
    return np.concatenate(outs, axis=0)


if __name__ == "__main__":
    nc = _get_nc()
    print("built + compiled OK")


# revision 9
# speedup vs baseline: 1.0373x; 1.0373x over previous
"""Causal single-head attention (S=4096, D=1024, fp32) on 8 TRN2 NeuronCores.

v3 (pair-split K/V projection + chunked pair-AllGather) restructured from
trace evidence:
 - kv exchange buffers are partition-major ([128, 16, 512]) so each K^T/V
   quarter loads with ONE DMA instead of 16 small SWDGE transfers.
 - K/V quarter 0 is projected BEFORE Q so AllGather 0 fires ~70us earlier.
 - scores are computed at full 512-column width (N=512, half the matmuls of
   the 256-wide version), with exp'd P^T tiles retained in SBUF for the whole
   block; A@V runs in two o-half passes over the retained P^T so the PSUM
   budget (8 banks) holds: 3 score banks + 4 output banks + 1 packed-sums.
"""

import numpy as np
import ml_dtypes

import concourse.bacc as bacc
import concourse.tile as tile
from concourse import mybir
from concourse.bass_utils import run_bass_kernel_spmd

S = 4096
D = 1024
NCORES = 8
ROWS = 512
P = 128
DC = 8
OT = 8
HALF = 2048
NQT = 4       # 512-row quarters per half (AG chunks)
NJT = 32
BF = mybir.dt.bfloat16
F32 = mybir.dt.float32
EXP = mybir.ActivationFunctionType.Exp
PAIRS = [[0, 1], [2, 3], [4, 5], [6, 7]]

bf16 = ml_dtypes.bfloat16


def build_nc():
    nc = bacc.Bacc(None, target_bir_lowering=False, debug=False)

    xq = nc.declare_dram_parameter("xqt", [D, ROWS], BF, isOutput=False)
    xk = nc.declare_dram_parameter("xkh", [D, HALF], BF, isOutput=False)
    xv = nc.declare_dram_parameter("xvh", [D, HALF], BF, isOutput=False)
    wq = nc.declare_dram_parameter("wqt", [D, D], BF, isOutput=False)
    wk = nc.declare_dram_parameter("wkt", [D, D], BF, isOutput=False)
    wv = nc.declare_dram_parameter("wvt", [D, D], BF, isOutput=False)
    msk = nc.declare_dram_parameter("mask01", [NJT, P, ROWS], BF, isOutput=False)
    out = nc.declare_dram_parameter("out", [ROWS, D], F32, isOutput=True)

    # partition-major exchange buffers: kvin[t][p, e, j]: e 0..7 = kT o_hi
    # tiles ([o_lo, j] per e), e 8..15 = V (jh*2+ob -> [j_lo, o]).
    kvin = [nc.dram_tensor(f"kvin{t}", [P, 16, 512], BF) for t in range(NQT)]
    kvout = [nc.dram_tensor(f"kvout{t}", [2 * P, 16, 512], BF) for t in range(NQT)]

    with tile.TileContext(nc) as tc:
        with tc.tile_pool(name="persist", bufs=1) as persist:
            ones = persist.tile([P, 16], BF, tag="ones", name="ones")
            nc.vector.memset(ones[:], 1.0)
            zbias = persist.tile([P, 1], F32, tag="zbias", name="zbias")
            nc.vector.memset(zbias[:], 0.0)
            qT = [persist.tile([P, ROWS], BF, tag=f"qT{t}", name=f"qT{t}") for t in range(OT)]
            # P^T tiles for the whole block and V quarters, retained for pass 2
            ptall = [persist.tile([P, ROWS], BF, tag=f"pt{j}", name=f"pt{j}") for j in range(NJT)]
            vtall = [persist.tile([P, 4, 512], BF, tag=f"vt{q}", name=f"vt{q}") for q in range(8)]

            with (
                tc.tile_pool(name="proj", bufs=1) as kp,
                tc.tile_pool(name="stg", bufs=6) as stg,
                tc.tile_pool(name="xs", bufs=2) as xs,
                tc.tile_pool(name="pps", bufs=4, space="PSUM") as pps,
            ):
                wk_t = [kp.tile([P, D], BF, tag=f"wk{d_}", name=f"wk{d_}") for d_ in range(DC)]
                wv_t = [kp.tile([P, D], BF, tag=f"wv{d_}", name=f"wv{d_}") for d_ in range(DC)]
                for d_ in range(DC):
                    nc.sync.dma_start(out=wk_t[d_][:], in_=wk[d_ * P:(d_ + 1) * P, :])
                    nc.sync.dma_start(out=wv_t[d_][:], in_=wv[d_ * P:(d_ + 1) * P, :])
                xk_t = {}
                xv_t = {}

                def load_x_quarter(t):
                    for d_ in range(DC):
                        xk_t[t, d_] = xs.tile([P, 512], BF, tag=f"xk{d_}", name=f"xk{d_}")
                        nc.sync.dma_start(out=xk_t[t, d_][:], in_=xk[d_ * P:(d_ + 1) * P, t * 512:(t + 1) * 512])
                        xv_t[t, d_] = xs.tile([P, 512], BF, tag=f"xv{d_}", name=f"xv{d_}")
                        nc.sync.dma_start(out=xv_t[t, d_][:], in_=xv[d_ * P:(d_ + 1) * P, t * 512:(t + 1) * 512])

                def kv_quarter(t):
                    for ohi in range(OT):
                        ps = pps.tile([P, 512], F32, tag="pp", name="ppk")
                        for d_ in range(DC):
                            nc.tensor.matmul(
                                ps[:],
                                lhsT=wk_t[d_][:, ohi * P:(ohi + 1) * P],
                                rhs=xk_t[t, d_][:],
                                start=(d_ == 0),
                                stop=(d_ == DC - 1),
                            )
                        sg = stg.tile([P, 512], BF, tag="sg", name="sg")
                        nc.vector.tensor_copy(sg[:], ps[:])
                        nc.gpsimd.dma_start(out=kvin[t][:, ohi, :], in_=sg[:])
                    for jh in range(4):
                        for ob in range(2):
                            ps = pps.tile([P, 512], F32, tag="pp", name="ppv")
                            for d_ in range(DC):
                                nc.tensor.matmul(
                                    ps[:],
                                    lhsT=xv_t[t, d_][:, jh * P:(jh + 1) * P],
                                    rhs=wv_t[d_][:, ob * 512:(ob + 1) * 512],
                                    start=(d_ == 0),
                                    stop=(d_ == DC - 1),
                                )
                            sg = stg.tile([P, 512], BF, tag="sg", name="sg")
                            nc.vector.tensor_copy(sg[:], ps[:])
                            nc.gpsimd.dma_start(out=kvin[t][:, 8 + ob * 4 + jh, :], in_=sg[:])
                    nc.gpsimd.collective_compute(
                        "AllGather",
                        mybir.AluOpType.bypass,
                        replica_groups=PAIRS,
                        ins=[kvin[t][:].opt()],
                        outs=[kvout[t][:].opt()],
                    )

                # quarters 0,1 first so AG0/AG1 fire early, then Q, then 2,3
                load_x_quarter(0)
                kv_quarter(0)
                load_x_quarter(1)
                kv_quarter(1)

                with tc.tile_pool(name="qph", bufs=1) as qp:
                    xq_t = [qp.tile([P, ROWS], BF, tag=f"xq{d_}", name=f"xq{d_}") for d_ in range(DC)]
                    wq_t = [qp.tile([P, D], BF, tag=f"wq{d_}", name=f"wq{d_}") for d_ in range(DC)]
                    for d_ in range(DC):
                        nc.sync.dma_start(out=xq_t[d_][:], in_=xq[d_ * P:(d_ + 1) * P, :])
                        nc.sync.dma_start(out=wq_t[d_][:], in_=wq[d_ * P:(d_ + 1) * P, :])
                    for t in range(OT):
                        ps = pps.tile([P, ROWS], F32, tag="pp", name="ppq")
                        for d_ in range(DC):
                            nc.tensor.matmul(
                                ps[:],
                                lhsT=wq_t[d_][:, t * P:(t + 1) * P],
                                rhs=xq_t[d_][:],
                                start=(d_ == 0),
                                stop=(d_ == DC - 1),
                            )
                        nc.vector.tensor_copy(qT[t][:], ps[:])

                for t in range(2, NQT):
                    load_x_quarter(t)
                    kv_quarter(t)

            # ---- Attention: pass 1 (scores + exp + mask + AV o-half 0 + sums),
            #      pass 2 (AV o-half 1 from retained P^T). ----
            with (
                tc.tile_pool(name="kvs", bufs=4) as kvs,
                tc.tile_pool(name="att", bufs=8) as ap,
                tc.tile_pool(name="att_out", bufs=4) as op,
                tc.tile_pool(name="sps", bufs=3, space="PSUM") as spsum,
                tc.tile_pool(name="ops", bufs=1, space="PSUM") as opsum,
            ):
                ops0 = {i: opsum.tile([P, 512], F32, tag=f"oa{i}", name=f"oa{i}") for i in range(4)}
                sums_bank = opsum.tile([P, 64], F32, tag="sums", name="sums")
                for qtr in range(8):
                    t, g = qtr // 2, qtr % 2
                    ktq = kvs.tile([P, OT, 512], BF, tag="ktq", name="ktq")
                    nc.sync.dma_start(out=ktq[:], in_=kvout[t][g * P:(g + 1) * P, 0:8, :])
                    vt0 = kvs.tile([P, 4, 512], BF, tag="vt0", name="vt0")
                    nc.sync.dma_start(out=vt0[:], in_=kvout[t][g * P:(g + 1) * P, 8:12, :])
                    nc.sync.dma_start(out=vtall[qtr][:], in_=kvout[t][g * P:(g + 1) * P, 12:16, :])
                    for jl in range(4):
                        jt = qtr * 4 + jl
                        sp = spsum.tile([P, ROWS], F32, tag="sps", name="sps")
                        for oc in range(OT):
                            nc.tensor.matmul(
                                sp[:],
                                lhsT=ktq[:, oc, jl * P:(jl + 1) * P],
                                rhs=qT[oc][:],
                                start=(oc == 0),
                                stop=(oc == OT - 1),
                            )
                        pt = ptall[jt]
                        nc.scalar.activation(pt[:], sp[:], EXP, bias=zbias[:])
                        mt = ap.tile([P, ROWS], BF, tag="mt", name="mt")
                        nc.gpsimd.dma_start(out=mt[:], in_=msk[jt, :, :])
                        nc.vector.tensor_mul(pt[:], pt[:], mt[:])
                        for isub in range(4):
                            pslice = pt[:, isub * P:(isub + 1) * P]
                            nc.tensor.matmul(
                                ops0[isub][:],
                                lhsT=pslice,
                                rhs=vt0[:, jl, :],
                                start=(jt == 0),
                                stop=(jt == NJT - 1),
                            )
                            nc.tensor.matmul(
                                sums_bank[:, isub * 16:(isub + 1) * 16],
                                lhsT=pslice,
                                rhs=ones[:],
                                start=(jt == 0 and isub == 0),
                                stop=(jt == NJT - 1 and isub == 3),
                                skip_group_check=True,
                            )
                recs = []
                for isub in range(4):
                    ssb = op.tile([P, 1], F32, tag="ssb", name="ssb")
                    nc.vector.tensor_copy(ssb[:], sums_bank[:, isub * 16:isub * 16 + 1])
                    rec = op.tile([P, 1], F32, tag=f"rec{isub}", name=f"rec{isub}")
                    nc.vector.reciprocal(rec[:], ssb[:])
                    recs.append(rec)
                    osb = op.tile([P, 512], F32, tag="osb", name="osb")
                    nc.vector.tensor_scalar_mul(osb[:], ops0[isub][:], recs[isub][:])
                    nc.sync.dma_start(out=out[isub * P:(isub + 1) * P, 0:512], in_=osb[:])
                # pass 2: o columns 512..1023 from retained P^T
                ops1 = {i: opsum.tile([P, 512], F32, tag=f"oa{i}", name=f"ob{i}") for i in range(4)}
                for qtr in range(8):
                    for jl in range(4):
                        jt = qtr * 4 + jl
                        for isub in range(4):
                            nc.tensor.matmul(
                                ops1[isub][:],
                                lhsT=ptall[jt][:, isub * P:(isub + 1) * P],
                                rhs=vtall[qtr][:, jl, :],
                                start=(jt == 0),
                                stop=(jt == NJT - 1),
                            )
                for isub in range(4):
                    osb = op.tile([P, 512], F32, tag="osb", name="osb")
                    nc.vector.tensor_scalar_mul(osb[:], ops1[isub][:], recs[isub][:])
                    nc.sync.dma_start(out=out[isub * P:(isub + 1) * P, 512:1024], in_=osb[:])
    return nc


_CACHE = {}


def _get_nc():
    if "nc" not in _CACHE:
        nc = build_nc()
        nc.compile()
        _CACHE["nc"] = nc
    return _CACHE["nc"]


def build_in_maps(inputs):
    x_q = np.asarray(inputs["encodings_for_q"], dtype=np.float32)
    x_k = np.asarray(inputs["encodings_for_k"], dtype=np.float32)
    x_v = np.asarray(inputs["encodings_for_v"], dtype=np.float32)
    W_q = np.asarray(inputs["W_q"], dtype=np.float32)
    W_k = np.asarray(inputs["W_k"], dtype=np.float32)
    W_v = np.asarray(inputs["W_v"], dtype=np.float32)

    wqt = np.ascontiguousarray(W_q.T).astype(bf16)
    wkt = np.ascontiguousarray(W_k.T / np.sqrt(D)).astype(bf16)
    wvt = np.ascontiguousarray(W_v.T).astype(bf16)

    causal = (np.arange(S)[:, None] <= np.arange(S)[None, :])

    in_maps = []
    for c in range(NCORES):
        rows = slice(ROWS * c, ROWS * (c + 1))
        h = slice(HALF * (c % 2), HALF * (c % 2 + 1))
        xqt_c = np.ascontiguousarray(x_q[rows].T).astype(bf16)
        xkh_c = np.ascontiguousarray(x_k[h].T).astype(bf16)
        xvh_c = np.ascontiguousarray(x_v[h].T).astype(bf16)
        m = causal[:, rows]  # [S, ROWS] in global j order
        mg = m.reshape(NJT, P, ROWS)
        order = []
        for qtr in range(8):
            t, g = qtr // 2, qtr % 2
            for jl in range(4):
                order.append(16 * g + 4 * t + jl)
        mask_c = np.ascontiguousarray(mg[order]).astype(bf16)
        in_maps.append(
            dict(
                xqt=xqt_c, xkh=xkh_c, xvh=xvh_c,
                wqt=wqt, wkt=wkt, wvt=wvt,
                mask01=mask_c,
            )
        )
    return in_maps


def kernel(**inputs):
    nc = _get_nc()
    in_maps = build_in_maps(inputs)
    res = run_bass_kernel_spmd(nc, in_maps, list(range(NCORES)))
    outs = [np.asarray(res.results[i]["out"], dtype=np.float32) for i in range(NCORES)]
    return np.concatenate(outs, axis=0)


# revision 10
# speedup vs baseline: 1.0647x; 1.0264x over previous
"""Causal single-head attention (S=4096, D=1024, fp32) on 8 TRN2 NeuronCores.

v6 (pair-split proj + chunked pair-AllGather + SBUF-accumulated A@V) with the
serialization fixed: attention quarters are emitted INSIDE the projection
stream (attn t0 between proj q2 and q3, the rest after), so the PE consumes
gathered chunks as they land instead of finishing all projections first.
Projection accumulation and score matmuls share one PSUM pool (4 banks) so
the total PSUM stays at 8: 4 shared + 3 A@V scratch + 1 packed sums.
"""

import numpy as np
import ml_dtypes

import concourse.bacc as bacc
import concourse.tile as tile
from concourse import mybir
from concourse.bass_utils import run_bass_kernel_spmd

S = 4096
D = 1024
NCORES = 8
ROWS = 512
P = 128
DC = 8
OT = 8
HALF = 2048
NQT = 4
NJT = 32
BF = mybir.dt.bfloat16
F32 = mybir.dt.float32
EXP = mybir.ActivationFunctionType.Exp
PAIRS = [[0, 1], [2, 3], [4, 5], [6, 7]]

bf16 = ml_dtypes.bfloat16


def build_nc():
    nc = bacc.Bacc(None, target_bir_lowering=False, debug=False)

    xq = nc.declare_dram_parameter("xqt", [D, ROWS], BF, isOutput=False)
    xk = nc.declare_dram_parameter("xkh", [D, HALF], BF, isOutput=False)
    xv = nc.declare_dram_parameter("xvh", [D, HALF], BF, isOutput=False)
    wq = nc.declare_dram_parameter("wqt", [D, D], BF, isOutput=False)
    wk = nc.declare_dram_parameter("wkt", [D, D], BF, isOutput=False)
    wv = nc.declare_dram_parameter("wvt", [D, D], BF, isOutput=False)
    msk = nc.declare_dram_parameter("mask01", [NJT, P, ROWS], BF, isOutput=False)
    out = nc.declare_dram_parameter("out", [ROWS, D], F32, isOutput=True)

    kvin = [nc.dram_tensor(f"kvin{t}", [P, 16, 512], BF) for t in range(NQT)]
    kvout = [nc.dram_tensor(f"kvout{t}", [2 * P, 16, 512], BF) for t in range(NQT)]

    with tile.TileContext(nc) as tc:
        with (
            tc.tile_pool(name="persist", bufs=1) as persist,
            tc.tile_pool(name="proj", bufs=1) as kp,
            tc.tile_pool(name="stg", bufs=6) as stg,
            tc.tile_pool(name="xs", bufs=2) as xs,
            tc.tile_pool(name="kvs", bufs=2) as kvs,
            tc.tile_pool(name="att", bufs=6) as ap,
            tc.tile_pool(name="att_out", bufs=3) as op,
            tc.tile_pool(name="pps", bufs=4, space="PSUM") as pps,
            tc.tile_pool(name="avs", bufs=3, space="PSUM") as avsum,
            tc.tile_pool(name="ops", bufs=1, space="PSUM") as opsum,
        ):
            ones = persist.tile([P, 16], BF, tag="ones", name="ones")
            nc.vector.memset(ones[:], 1.0)
            zbias = persist.tile([P, 1], F32, tag="zbias", name="zbias")
            nc.vector.memset(zbias[:], 0.0)
            qT = [persist.tile([P, ROWS], BF, tag=f"qT{t}", name=f"qT{t}") for t in range(OT)]
            acc = {}
            for isub in range(4):
                for ob in range(2):
                    acc[isub, ob] = persist.tile([P, 512], F32, tag=f"acc{isub}{ob}", name=f"acc{isub}{ob}")
                    nc.vector.memset(acc[isub, ob][:], 0.0)
            sums_bank = opsum.tile([P, 64], F32, tag="sums", name="sums")

            wk_t = [kp.tile([P, D], BF, tag=f"wk{d_}", name=f"wk{d_}") for d_ in range(DC)]
            wv_t = [kp.tile([P, D], BF, tag=f"wv{d_}", name=f"wv{d_}") for d_ in range(DC)]
            xk_t = {}
            xv_t = {}

            def load_x_quarter(t, k_first=False):
                for d_ in range(DC):
                    xk_t[t, d_] = xs.tile([P, 512], BF, tag=f"xk{d_}", name=f"xk{d_}")
                    nc.sync.dma_start(out=xk_t[t, d_][:], in_=xk[d_ * P:(d_ + 1) * P, t * 512:(t + 1) * 512])
                    if not k_first:
                        xv_t[t, d_] = xs.tile([P, 512], BF, tag=f"xv{d_}", name=f"xv{d_}")
                        nc.sync.dma_start(out=xv_t[t, d_][:], in_=xv[d_ * P:(d_ + 1) * P, t * 512:(t + 1) * 512])
                if k_first:
                    for d_ in range(DC):
                        xv_t[t, d_] = xs.tile([P, 512], BF, tag=f"xv{d_}", name=f"xv{d_}")
                        nc.sync.dma_start(out=xv_t[t, d_][:], in_=xv[d_ * P:(d_ + 1) * P, t * 512:(t + 1) * 512])

            def kv_quarter(t):
                for ohi in range(OT):
                    ps = pps.tile([P, 512], F32, tag="pp", name="ppk")
                    for d_ in range(DC):
                        nc.tensor.matmul(
                            ps[:],
                            lhsT=wk_t[d_][:, ohi * P:(ohi + 1) * P],
                            rhs=xk_t[t, d_][:],
                            start=(d_ == 0),
                            stop=(d_ == DC - 1),
                        )
                    sg = stg.tile([P, 512], BF, tag="sg", name="sg")
                    nc.vector.tensor_copy(sg[:], ps[:])
                    nc.sync.dma_start(out=kvin[t][:, ohi, :], in_=sg[:])
                for jh in range(4):
                    for ob in range(2):
                        ps = pps.tile([P, 512], F32, tag="pp", name="ppv")
                        for d_ in range(DC):
                            nc.tensor.matmul(
                                ps[:],
                                lhsT=xv_t[t, d_][:, jh * P:(jh + 1) * P],
                                rhs=wv_t[d_][:, ob * 512:(ob + 1) * 512],
                                start=(d_ == 0),
                                stop=(d_ == DC - 1),
                            )
                        sg = stg.tile([P, 512], BF, tag="sg", name="sg")
                        nc.vector.tensor_copy(sg[:], ps[:])
                        nc.sync.dma_start(out=kvin[t][:, 8 + ob * 4 + jh, :], in_=sg[:])
                nc.gpsimd.collective_compute(
                    "AllGather",
                    mybir.AluOpType.bypass,
                    replica_groups=PAIRS,
                    ins=[kvin[t][:].opt()],
                    outs=[kvout[t][:].opt()],
                )

            def attn_quarter(qtr):
                t, g = qtr // 2, qtr % 2
                ktq = kvs.tile([P, OT, 512], BF, tag="ktq", name="ktq")
                nc.gpsimd.dma_start(out=ktq[:], in_=kvout[t][g * P:(g + 1) * P, 0:8, :])
                vtq = kvs.tile([P, OT, 512], BF, tag="vtq", name="vtq")
                nc.gpsimd.dma_start(out=vtq[:], in_=kvout[t][g * P:(g + 1) * P, 8:16, :])
                ptq = []
                for jl in range(4):
                    jt = qtr * 4 + jl
                    sp = pps.tile([P, ROWS], F32, tag="pp", name="sps")
                    for oc in range(OT):
                        nc.tensor.matmul(
                            sp[:],
                            lhsT=ktq[:, oc, jl * P:(jl + 1) * P],
                            rhs=qT[oc][:],
                            start=(oc == 0),
                            stop=(oc == OT - 1),
                        )
                    pt = ap.tile([P, ROWS], BF, tag="pt", name="pt")
                    nc.scalar.activation(pt[:], sp[:], EXP, bias=zbias[:])
                    mt = ap.tile([P, ROWS], BF, tag="mt", name="mt")
                    nc.gpsimd.dma_start(out=mt[:], in_=msk[jt, :, :])
                    nc.vector.tensor_mul(pt[:], pt[:], mt[:])
                    ptq.append(pt)
                    for isub in range(4):
                        nc.tensor.matmul(
                            sums_bank[:, isub * 16:(isub + 1) * 16],
                            lhsT=pt[:, isub * P:(isub + 1) * P],
                            rhs=ones[:],
                            start=(jt == 0 and isub == 0),
                            stop=(jt == NJT - 1 and isub == 3),
                            skip_group_check=True,
                        )
                for isub in range(4):
                    for ob in range(2):
                        sc = avsum.tile([P, 512], F32, tag="avs", name="avs")
                        for jl in range(4):
                            nc.tensor.matmul(
                                sc[:],
                                lhsT=ptq[jl][:, isub * P:(isub + 1) * P],
                                rhs=vtq[:, ob * 4 + jl, :],
                                start=(jl == 0),
                                stop=(jl == 3),
                            )
                        nc.vector.tensor_add(acc[isub, ob][:], acc[isub, ob][:], sc[:])

            # ---- interleaved schedule: K weights + x_k q0 load first ----
            for d_ in range(DC):
                nc.sync.dma_start(out=wk_t[d_][:], in_=wk[d_ * P:(d_ + 1) * P, :])
            load_x_quarter(0, k_first=True)
            for d_ in range(DC):
                nc.sync.dma_start(out=wv_t[d_][:], in_=wv[d_ * P:(d_ + 1) * P, :])
            kv_quarter(0)
            load_x_quarter(1)
            kv_quarter(1)

            xq_t = [kp.tile([P, ROWS], BF, tag=f"xq{d_}", name=f"xq{d_}") for d_ in range(DC)]
            wq_t = [kp.tile([P, D], BF, tag=f"wq{d_}", name=f"wq{d_}") for d_ in range(DC)]
            for d_ in range(DC):
                nc.sync.dma_start(out=xq_t[d_][:], in_=xq[d_ * P:(d_ + 1) * P, :])
                nc.sync.dma_start(out=wq_t[d_][:], in_=wq[d_ * P:(d_ + 1) * P, :])
            for t in range(OT):
                ps = pps.tile([P, ROWS], F32, tag="pp", name="ppq")
                for d_ in range(DC):
                    nc.tensor.matmul(
                        ps[:],
                        lhsT=wq_t[d_][:, t * P:(t + 1) * P],
                        rhs=xq_t[d_][:],
                        start=(d_ == 0),
                        stop=(d_ == DC - 1),
                    )
                nc.vector.tensor_copy(qT[t][:], ps[:])

            load_x_quarter(2)
            kv_quarter(2)
            attn_quarter(0)
            load_x_quarter(3)
            kv_quarter(3)
            for qtr in range(1, 8):
                attn_quarter(qtr)

            for isub in range(4):
                ssb = op.tile([P, 1], F32, tag="ssb", name="ssb")
                nc.vector.tensor_copy(ssb[:], sums_bank[:, isub * 16:isub * 16 + 1])
                rec = op.tile([P, 1], F32, tag=f"rec{isub}", name=f"rec{isub}")
                nc.vector.reciprocal(rec[:], ssb[:])
                for ob in range(2):
                    osb = op.tile([P, 512], F32, tag="osb", name="osb")
                    nc.vector.tensor_scalar_mul(osb[:], acc[isub, ob][:], rec[:])
                    nc.sync.dma_start(out=out[isub * P:(isub + 1) * P, ob * 512:(ob + 1) * 512], in_=osb[:])
    return nc


_CACHE = {}


def _get_nc():
    if "nc" not in _CACHE:
        nc = build_nc()
        nc.compile()
        _CACHE["nc"] = nc
    return _CACHE["nc"]


def build_in_maps(inputs):
    x_q = np.asarray(inputs["encodings_for_q"], dtype=np.float32)
    x_k = np.asarray(inputs["encodings_for_k"], dtype=np.float32)
    x_v = np.asarray(inputs["encodings_for_v"], dtype=np.float32)
    W_q = np.asarray(inputs["W_q"], dtype=np.float32)
    W_k = np.asarray(inputs["W_k"], dtype=np.float32)
    W_v = np.asarray(inputs["W_v"], dtype=np.float32)

    wqt = np.ascontiguousarray(W_q.T).astype(bf16)
    wkt = np.ascontiguousarray(W_k.T / np.sqrt(D)).astype(bf16)
    wvt = np.ascontiguousarray(W_v.T).astype(bf16)

    causal = (np.arange(S)[:, None] <= np.arange(S)[None, :])

    in_maps = []
    for c in range(NCORES):
        rows = slice(ROWS * c, ROWS * (c + 1))
        h = slice(HALF * (c % 2), HALF * (c % 2 + 1))
        xqt_c = np.ascontiguousarray(x_q[rows].T).astype(bf16)
        xkh_c = np.ascontiguousarray(x_k[h].T).astype(bf16)
        xvh_c = np.ascontiguousarray(x_v[h].T).astype(bf16)
        m = causal[:, rows]
        mg = m.reshape(NJT, P, ROWS)
        order = []
        for qtr in range(8):
            t, g = qtr // 2, qtr % 2
            for jl in range(4):
                order.append(16 * g + 4 * t + jl)
        mask_c = np.ascontiguousarray(mg[order]).astype(bf16)
        in_maps.append(
            dict(
                xqt=xqt_c, xkh=xkh_c, xvh=xvh_c,
                wqt=wqt, wkt=wkt, wvt=wvt,
                mask01=mask_c,
            )
        )
    return in_maps


def kernel(**inputs):
    nc = _get_nc()
    in_maps = build_in_maps(inputs)
    res = run_bass_kernel_spmd(nc, in_maps, list(range(NCORES)))
    outs = [np.asarray(res.results[i]["out"], dtype=np.float32) for i in range(NCORES)]
    return np.concatenate(outs, axis=0)


# revision 11
# speedup vs baseline: 1.0886x; 1.0224x over previous
"""Causal single-head attention (S=4096, D=1024, fp32) on 8 TRN2 NeuronCores.

v6 (pair-split proj + chunked pair-AllGather + SBUF-accumulated A@V) with the
serialization fixed: attention quarters are emitted INSIDE the projection
stream (attn t0 between proj q2 and q3, the rest after), so the PE consumes
gathered chunks as they land instead of finishing all projections first.
Projection accumulation and score matmuls share one PSUM pool (4 banks) so
the total PSUM stays at 8: 4 shared + 3 A@V scratch + 1 packed sums.
"""

import numpy as np
import ml_dtypes

import concourse.bacc as bacc
import concourse.tile as tile
from concourse import mybir
from concourse.bass_utils import run_bass_kernel_spmd

S = 4096
D = 1024
NCORES = 8
ROWS = 512
P = 128
DC = 8
OT = 8
HALF = 2048
NQT = 4
NJT = 32
BF = mybir.dt.bfloat16
F32 = mybir.dt.float32
EXP = mybir.ActivationFunctionType.Exp
PAIRS = [[0, 1], [2, 3], [4, 5], [6, 7]]

bf16 = ml_dtypes.bfloat16


def build_nc():
    nc = bacc.Bacc(None, target_bir_lowering=False, debug=False)

    xq = nc.declare_dram_parameter("xqt", [D, ROWS], BF, isOutput=False)
    xk = nc.declare_dram_parameter("xkh", [D, HALF], BF, isOutput=False)
    xv = nc.declare_dram_parameter("xvh", [D, HALF], BF, isOutput=False)
    wq = nc.declare_dram_parameter("wqt", [D, D], BF, isOutput=False)
    wk = nc.declare_dram_parameter("wkt", [D, D], BF, isOutput=False)
    wv = nc.declare_dram_parameter("wvt", [D, D], BF, isOutput=False)
    msk = nc.declare_dram_parameter("mask01", [NJT, P, ROWS], BF, isOutput=False)
    out = nc.declare_dram_parameter("out", [ROWS, D], F32, isOutput=True)

    kvin = [nc.dram_tensor(f"kvin{t}", [P, 16, 512], BF) for t in range(NQT)]
    kvout = [nc.dram_tensor(f"kvout{t}", [2 * P, 16, 512], BF) for t in range(NQT)]


    with tile.TileContext(nc) as tc:
        with (
            tc.tile_pool(name="persist", bufs=1) as persist,
            tc.tile_pool(name="proj", bufs=1) as kp,
            tc.tile_pool(name="stg", bufs=6) as stg,
            tc.tile_pool(name="xs", bufs=2) as xs,
            tc.tile_pool(name="kvs", bufs=2) as kvs,
            tc.tile_pool(name="att", bufs=6) as ap,
            tc.tile_pool(name="att_out", bufs=3) as op,
            tc.tile_pool(name="pps", bufs=5, space="PSUM") as pps,
            tc.tile_pool(name="avs", bufs=2, space="PSUM") as avsum,
            tc.tile_pool(name="ops", bufs=1, space="PSUM") as opsum,
        ):
            ones = persist.tile([P, 16], BF, tag="ones", name="ones")
            nc.vector.memset(ones[:], 1.0)
            zbias = persist.tile([P, 1], F32, tag="zbias", name="zbias")
            nc.vector.memset(zbias[:], 0.0)
            qT = [persist.tile([P, ROWS], BF, tag=f"qT{t}", name=f"qT{t}") for t in range(OT)]
            acc = {}
            for isub in range(4):
                for ob in range(2):
                    acc[isub, ob] = persist.tile([P, 512], F32, tag=f"acc{isub}{ob}", name=f"acc{isub}{ob}")
                    nc.vector.memset(acc[isub, ob][:], 0.0)
            sums_bank = opsum.tile([P, 64], F32, tag="sums", name="sums")

            wk_t = [kp.tile([P, D], BF, tag=f"wk{d_}", name=f"wk{d_}") for d_ in range(DC)]
            wv_t = [kp.tile([P, D], BF, tag=f"wv{d_}", name=f"wv{d_}") for d_ in range(DC)]
            xk_t = {}
            xv_t = {}

            def load_x_quarter(t, k_first=False):
                for d_ in range(DC):
                    xk_t[t, d_] = xs.tile([P, 512], BF, tag=f"xk{d_}", name=f"xk{d_}")
                    nc.sync.dma_start(out=xk_t[t, d_][:], in_=xk[d_ * P:(d_ + 1) * P, t * 512:(t + 1) * 512])
                    if not k_first:
                        xv_t[t, d_] = xs.tile([P, 512], BF, tag=f"xv{d_}", name=f"xv{d_}")
                        nc.sync.dma_start(out=xv_t[t, d_][:], in_=xv[d_ * P:(d_ + 1) * P, t * 512:(t + 1) * 512])
                if k_first:
                    for d_ in range(DC):
                        xv_t[t, d_] = xs.tile([P, 512], BF, tag=f"xv{d_}", name=f"xv{d_}")
                        nc.sync.dma_start(out=xv_t[t, d_][:], in_=xv[d_ * P:(d_ + 1) * P, t * 512:(t + 1) * 512])

            def kv_quarter(t):
                for ohi in range(OT):
                    ps = pps.tile([P, 512], F32, tag="pp", name="ppk")
                    for d_ in range(DC):
                        nc.tensor.matmul(
                            ps[:],
                            lhsT=wk_t[d_][:, ohi * P:(ohi + 1) * P],
                            rhs=xk_t[t, d_][:],
                            start=(d_ == 0),
                            stop=(d_ == DC - 1),
                        )
                    sg = stg.tile([P, 512], BF, tag="sg", name="sg")
                    nc.scalar.copy(sg[:], ps[:])
                    nc.gpsimd.dma_start(out=kvin[t][:, ohi, :], in_=sg[:])
                for jh in range(4):
                    for ob in range(2):
                        ps = pps.tile([P, 512], F32, tag="pp", name="ppv")
                        for d_ in range(DC):
                            nc.tensor.matmul(
                                ps[:],
                                lhsT=xv_t[t, d_][:, jh * P:(jh + 1) * P],
                                rhs=wv_t[d_][:, ob * 512:(ob + 1) * 512],
                                start=(d_ == 0),
                                stop=(d_ == DC - 1),
                            )
                        sg = stg.tile([P, 512], BF, tag="sg", name="sg")
                        nc.scalar.copy(sg[:], ps[:])
                        nc.gpsimd.dma_start(out=kvin[t][:, 8 + ob * 4 + jh, :], in_=sg[:])
                nc.gpsimd.collective_compute(
                    "AllGather",
                    mybir.AluOpType.bypass,
                    replica_groups=PAIRS,
                    ins=[kvin[t][:].opt()],
                    outs=[kvout[t][:].opt()],
                )

            def attn_quarter(qtr):
                t, g = qtr // 2, qtr % 2
                ktq = kvs.tile([P, OT, 512], BF, tag="ktq", name="ktq")
                nc.scalar.dma_start(out=ktq[:], in_=kvout[t][g * P:(g + 1) * P, 0:8, :])
                vtq = kvs.tile([P, OT, 512], BF, tag="vtq", name="vtq")
                nc.scalar.dma_start(out=vtq[:], in_=kvout[t][g * P:(g + 1) * P, 8:16, :])
                ptq = []
                for jl in range(4):
                    jt = qtr * 4 + jl
                    sp = pps.tile([P, ROWS], F32, tag="pp", name="sps")
                    for oc in range(OT):
                        nc.tensor.matmul(
                            sp[:],
                            lhsT=ktq[:, oc, jl * P:(jl + 1) * P],
                            rhs=qT[oc][:],
                            start=(oc == 0),
                            stop=(oc == OT - 1),
                        )
                    pt = ap.tile([P, ROWS], BF, tag="pt", name="pt")
                    nc.scalar.activation(pt[:], sp[:], EXP, bias=zbias[:])
                    mt = ap.tile([P, ROWS], BF, tag="mt", name="mt")
                    nc.gpsimd.dma_start(out=mt[:], in_=msk[jt, :, :])
                    nc.vector.tensor_mul(pt[:], pt[:], mt[:])
                    ptq.append(pt)
                    for isub in range(4):
                        nc.tensor.matmul(
                            sums_bank[:, isub * 16:(isub + 1) * 16],
                            lhsT=pt[:, isub * P:(isub + 1) * P],
                            rhs=ones[:],
                            start=(jt == 0 and isub == 0),
                            stop=(jt == NJT - 1 and isub == 3),
                            skip_group_check=True,
                        )
                for isub in range(4):
                    for ob in range(2):
                        sc = avsum.tile([P, 512], F32, tag="avs", name="avs")
                        for jl in range(4):
                            nc.tensor.matmul(
                                sc[:],
                                lhsT=ptq[jl][:, isub * P:(isub + 1) * P],
                                rhs=vtq[:, ob * 4 + jl, :],
                                start=(jl == 0),
                                stop=(jl == 3),
                            )
                        nc.vector.tensor_add(acc[isub, ob][:], acc[isub, ob][:], sc[:])

            # ---- interleaved schedule: K weights + x_k q0 load first ----
            for d_ in range(DC):
                nc.sync.dma_start(out=wk_t[d_][:], in_=wk[d_ * P:(d_ + 1) * P, :])
            load_x_quarter(0, k_first=True)
            for d_ in range(DC):
                nc.sync.dma_start(out=wv_t[d_][:], in_=wv[d_ * P:(d_ + 1) * P, :])
            kv_quarter(0)
            load_x_quarter(1)
            kv_quarter(1)

            xq_t = [kp.tile([P, ROWS], BF, tag=f"xq{d_}", name=f"xq{d_}") for d_ in range(DC)]
            wq_t = [kp.tile([P, D], BF, tag=f"wq{d_}", name=f"wq{d_}") for d_ in range(DC)]
            for d_ in range(DC):
                nc.sync.dma_start(out=xq_t[d_][:], in_=xq[d_ * P:(d_ + 1) * P, :])
                nc.sync.dma_start(out=wq_t[d_][:], in_=wq[d_ * P:(d_ + 1) * P, :])
            for t in range(OT):
                ps = pps.tile([P, ROWS], F32, tag="pp", name="ppq")
                for d_ in range(DC):
                    nc.tensor.matmul(
                        ps[:],
                        lhsT=wq_t[d_][:, t * P:(t + 1) * P],
                        rhs=xq_t[d_][:],
                        start=(d_ == 0),
                        stop=(d_ == DC - 1),
                    )
                nc.vector.tensor_copy(qT[t][:], ps[:])

            load_x_quarter(2)
            kv_quarter(2)
            attn_quarter(0)
            load_x_quarter(3)
            kv_quarter(3)
            for qtr in range(1, 8):
                attn_quarter(qtr)

            for isub in range(4):
                ssb = op.tile([P, 1], F32, tag="ssb", name="ssb")
                nc.vector.tensor_copy(ssb[:], sums_bank[:, isub * 16:isub * 16 + 1])
                rec = op.tile([P, 1], F32, tag=f"rec{isub}", name=f"rec{isub}")
                nc.vector.reciprocal(rec[:], ssb[:])
                for ob in range(2):
                    osb = op.tile([P, 512], F32, tag="osb", name="osb")
                    nc.vector.tensor_scalar_mul(osb[:], acc[isub, ob][:], rec[:])
                    nc.sync.dma_start(out=out[isub * P:(isub + 1) * P, ob * 512:(ob + 1) * 512], in_=osb[:])
    return nc


_CACHE = {}


def _get_nc():
    if "nc" not in _CACHE:
        nc = build_nc()
        nc.compile()
        _CACHE["nc"] = nc
    return _CACHE["nc"]


def build_in_maps(inputs):
    x_q = np.asarray(inputs["encodings_for_q"], dtype=np.float32)
    x_k = np.asarray(inputs["encodings_for_k"], dtype=np.float32)
    x_v = np.asarray(inputs["encodings_for_v"], dtype=np.float32)
    W_q = np.asarray(inputs["W_q"], dtype=np.float32)
    W_k = np.asarray(inputs["W_k"], dtype=np.float32)
    W_v = np.asarray(inputs["W_v"], dtype=np.float32)

    wqt = np.ascontiguousarray(W_q.T).astype(bf16)
    wkt = np.ascontiguousarray(W_k.T / np.sqrt(D)).astype(bf16)
    wvt = np.ascontiguousarray(W_v.T).astype(bf16)

    causal = (np.arange(S)[:, None] <= np.arange(S)[None, :])

    in_maps = []
    for c in range(NCORES):
        rows = slice(ROWS * c, ROWS * (c + 1))
        h = slice(HALF * (c % 2), HALF * (c % 2 + 1))
        xqt_c = np.ascontiguousarray(x_q[rows].T).astype(bf16)
        xkh_c = np.ascontiguousarray(x_k[h].T).astype(bf16)
        xvh_c = np.ascontiguousarray(x_v[h].T).astype(bf16)
        m = causal[:, rows]
        mg = m.reshape(NJT, P, ROWS)
        order = []
        for qtr in range(8):
            t, g = qtr // 2, qtr % 2
            for jl in range(4):
                order.append(16 * g + 4 * t + jl)
        mask_c = np.ascontiguousarray(mg[order]).astype(bf16)
        in_maps.append(
            dict(
                xqt=xqt_c, xkh=xkh_c, xvh=xvh_c,
                wqt=wqt, wkt=wkt, wvt=wvt,
                mask01=mask_c,
            )
        )
    return in_maps


def kernel(**inputs):
    nc = _get_nc()
    in_maps = build_in_maps(inputs)
    res = run_bass_kernel_spmd(nc, in_maps, list(range(NCORES)))
    outs = [np.asarray(res.results[i]["out"], dtype=np.float32) for i in range(NCORES)]
    return np.concatenate(outs, axis=0)
